# revision 5
# baseline (speedup 1.0000x reference)
"""Trainium2 Bass kernel for the interaction-network GNN (nn_Physics_7370163880185).

Reference computation (per batch element b, objects i=0..15, D=256):
  trans  = MLP_t(objs)                              # 256->512->512->256, relu x3
  pair(i,j) = concat(objs_i, objs_j)                # [512]
  inter  = MLP_i(pair)                              # 512->512->512->256, relu x3
  out    = trans + sum_{j != i} inter(i,j) + objs

Sharding: data-parallel over batch B=512 across 8 cores (64 per core).

Kernel strategy (per core):
  * Feature-on-partition layout: activations are [feat, rows] with
    rows = (n, b) flattened; matmul(out, lhsT=W[k,m], rhs=xT[k, rows])
    chains layers without transposes.
  * Interaction layer 1 is split: concat(a,b) @ iW1 = a @ iW1[:256] + b @ iW1[256:],
    so U = objs @ iW1[:256] + ib1 and V = objs @ iW1[256:] are computed once on
    N*B rows; h1(i,j) = relu(U_i + V_j) is a broadcast add.
  * Phase 2 iterates over rotations s=1..15: rotation s computes
    inter(i, (i+s) mod 16) for all i at once in (i, b) column order, so the
    masked diagonal is never computed, h1 = relu(U + roll(V, s)) is two
    dense shifted adds (no broadcast), and the j-sum is a running
    accumulation acc += relu(L3 + ib3) (no masking, no tree reduce).
  * Mixed precision: U/V/L1 and the interaction L3 run in bf16 (fp32 PSUM).
    The transition MLP and the L2 matmuls of the last NROT_F8 rotations run
    in fp8-e4m3 with perf_mode=DoubleRow (2 k-tiles contracted per pass,
    ~2x per-matmul), keeping worst-case output error ~1.2e-2 vs the 2e-2
    gate (budget measured elementwise against the fp32 oracle).
  * Engine balance: DVE does the h1 adds + relu/fp8-convert and, in the
    fp8 rotations (where the PE window shrinks below the scalar drain
    time), the L3 drains via tensor_scalar(psum, +bias, max 0). The
    scalar engine drains everything else; gpsimd owns acc accumulation.
  * The t-MLP's 2nd/3rd layers are emitted inside the rotation stream
    (after rotations 1 and 3) where the scalar engine has slack; phase 1
    proper is only U/V/L1 and the t-MLP first layer.
  * Weights/biases are packed host-side; fp8 DoubleRow weights use the
    [128, kp*2*fout] layout with col = kp*2*fout + m*256 + h*128 + c where
    the contracted feature is f = kp*256 + h*128 + p.
"""

import numpy as np

import concourse.bass as bass
import concourse.mybir as mybir
import concourse.tile as tile
from concourse import bacc
from concourse.bass_utils import run_bass_kernel_spmd

N = 16
B = 512
D = 256
NCORES = 8
BL = B // NCORES          # 64 batch rows per core
ROWS = N * BL             # 1024 (n, b) rows per core
PT = 128                  # partition tile
NT = 512                  # matmul free-dim tile (one PSUM bank of fp32)

NROT_F8 = 8               # rotations (of 15) whose L2 runs in fp8 DoubleRow

F32 = mybir.dt.float32
BF16 = mybir.dt.bfloat16
FP8 = mybir.dt.float8e4
RELU = mybir.ActivationFunctionType.Relu
IDENT = mybir.ActivationFunctionType.Identity
ADD = mybir.AluOpType.add
MAX = mybir.AluOpType.max
DRMODE = mybir.MatmulPerfMode.DoubleRow

# bf16 weights, packed [128, nk*fout] host-side (k-tiles side by side)
WEIGHT_SHAPES = {
    "iW1": (256, 512), "iW1b": (256, 512),
    "iW2": (512, 512), "iW3": (512, 256),
}
# fp8 DoubleRow weights, packed [128, nkp*2*fout]
DR_WEIGHT_SHAPES = {
    "tW1d": (256, 512), "tW2d": (512, 512), "tW3d": (512, 256),
    "iW2d": (512, 512),
}
# biases packed into one [128, 20] tensor, in this column order
BIAS_COLS = {"tb1": (0, 4), "tb2": (4, 8), "tb3": (8, 10),
             "ib1": (10, 14), "ib2": (14, 18), "ib3": (18, 20)}


def _build_body(nc, tc, prm, ctx):
    cpool = ctx.enter_context(tc.tile_pool(name="const", bufs=1))
    wpool = ctx.enter_context(tc.tile_pool(name="work", bufs=2))
    ppool = ctx.enter_context(tc.tile_pool(name="psum", bufs=2, space="PSUM"))

    # ---- inputs: packed SBUF layout, per-k-tile DMAs (parallel rings),
    # in first-use order ---------------------------------------------------
    biases = cpool.tile([PT, 20], F32, tag="biases", bufs=1, name="biases")
    nc.sync.dma_start(out=biases, in_=prm["biases"][:, :])

    objsb = cpool.tile([PT, 2 * ROWS], BF16, tag="objsb", bufs=1, name="objsb")
    for p in range(2):
        nc.sync.dma_start(out=objsb[:, p * ROWS:(p + 1) * ROWS],
                          in_=prm["objsb"][:, p * ROWS:(p + 1) * ROWS])
    objsT_b = [objsb[:, p * ROWS:(p + 1) * ROWS] for p in range(2)]

    objs8 = cpool.tile([PT, 2 * ROWS], FP8, tag="objs8", bufs=1, name="objs8")
    nc.sync.dma_start(out=objs8, in_=prm["objs8"][:, :])

    w_sb = {}

    def load_weights(*names):
        for wname in names:
            if wname in WEIGHT_SHAPES:
                fin, fout = WEIGHT_SHAPES[wname]
                dt = BF16
                ncols = (fin // PT) * fout
            else:
                fin, fout = DR_WEIGHT_SHAPES[wname]
                dt = FP8
                ncols = (fin // 256) * 2 * fout
            t = cpool.tile([PT, ncols], dt, tag=wname, bufs=1, name=wname)
            for c0 in range(0, ncols, fout):
                nc.sync.dma_start(out=t[:, c0:c0 + fout],
                                  in_=prm[wname][:, c0:c0 + fout])
            w_sb[wname] = t

    load_weights("iW1", "iW1b", "tW1d", "iW2", "tW2d", "iW2d", "tW3d", "iW3")

    def wslice(wname, k, m):
        fout = WEIGHT_SHAPES[wname][1]
        return w_sb[wname][:, k * fout + m * PT:k * fout + (m + 1) * PT]

    def wdr(wname, kp, m):
        fout = DR_WEIGHT_SHAPES[wname][1]
        base = kp * 2 * fout + m * 256
        return w_sb[wname][:, base:base + 256].rearrange("p (h c) -> p h c", h=2)

    def bias(bname, m):
        lo, hi = BIAS_COLS[bname]
        assert lo + m < hi
        return biases[:, lo + m:lo + m + 1]

    objsf = cpool.tile([PT, 2 * ROWS], F32, tag="objsf", bufs=1, name="objsf")
    for p in range(2):
        nc.sync.dma_start(out=objsf[:, p * ROWS:(p + 1) * ROWS],
                          in_=prm["objs"][:, p * ROWS:(p + 1) * ROWS])
    objsT_f = [objsf[:, p * ROWS:(p + 1) * ROWS] for p in range(2)]

    # ---- generic bf16 dense layer over the full ROWS ------------------
    # Pairs of column-chunks share each 128-col weight tile so LDWEIGHTS
    # amortizes over 2 matmuls, and consecutive MMs hit different banks.
    def layer(wname, rhs, drain):
        fin, fout = WEIGHT_SHAPES[wname]
        nk = fin // PT
        nm = fout // PT
        ncols = rhs[0].shape[-1]
        mc = [(m, c) for m in range(nm) for c in range(ncols // NT)]
        for g in range(0, len(mc), 2):
            grp = mc[g:g + 2]
            pss = [ppool.tile([PT, NT], F32, tag="ps", bufs=8,
                              name=f"ps_{wname}_{m}_{c}") for (m, c) in grp]
            for k in range(nk):
                for (m, c), ps in zip(grp, pss):
                    nc.tensor.matmul(
                        ps, wslice(wname, k, m),
                        rhs[k][:, c * NT:(c + 1) * NT],
                        start=(k == 0), stop=(k == nk - 1))
            for (m, c), ps in zip(grp, pss):
                drain(m, c, ps)

    # ---- generic fp8 DoubleRow layer: rhs is a [PT, nkp*2*ROWS] fp8 tile
    # with k-major column layout (col = k*ROWS + r, k = kp*2 + h) ----------
    def layer_dr(wname, rhs, drain):
        fin, fout = DR_WEIGHT_SHAPES[wname]
        nkp = fin // 256
        nm = fout // PT
        r4 = rhs.rearrange("p (kp h r) -> p kp h r", kp=nkp, h=2)
        for m in range(nm):
            pss = [ppool.tile([PT, NT], F32, tag="ps", bufs=8,
                              name=f"ps_{wname}_{m}_{c}") for c in range(2)]
            for kp in range(nkp):
                for c in range(2):
                    nc.tensor.matmul(
                        pss[c], wdr(wname, kp, m),
                        r4[:, kp, :, c * NT:(c + 1) * NT],
                        start=(kp == 0), stop=(kp == nkp - 1),
                        perf_mode=DRMODE)
            for c in range(2):
                drain(m, c, pss[c])

    def persist(tag, n_tiles, dt=BF16, cols=ROWS):
        return [cpool.tile([PT, cols], dt, tag=f"{tag}_{m}", bufs=1, name=f"{tag}_{m}")
                for m in range(n_tiles)]

    # ---- phase 1: U, V (bf16) + trans MLP layer 1 (fp8 DR) ---------------
    U = persist("U", 4)       # bf16(objs @ iW1[:256] + ib1)
    V = persist("V", 4)       # bf16(objs @ iW1[256:])
    t1f8 = cpool.tile([PT, 4 * ROWS], FP8, tag="t1f8", bufs=1, name="t1f8")
    t2f8 = cpool.tile([PT, 4 * ROWS], FP8, tag="t2f8", bufs=1, name="t2f8")
    t3 = persist("t3", 2, F32)   # becomes S = t3 + objs after in-place add

    # h1 for rotation s: h1 = relu(U + roll(V, s)) — two dense shifted adds
    # plus a relu pass, all on DVE. bf16 rotations keep 4x [PT, ROWS] bf16
    # tiles; fp8 rotations write one [PT, 4*ROWS] fp8 tile (k-major).
    def make_h1(s):
        sp = (N - s) * BL     # split point: i < N-s reads V at +s*BL
        h1r = [wpool.tile([PT, ROWS], BF16, tag=f"h1r_{p}", bufs=3,
                          name=f"h1r_{s}_{p}") for p in range(4)]
        for p in range(4):
            nc.vector.tensor_add(h1r[p][:, 0:sp], U[p][:, 0:sp],
                                 V[p][:, s * BL:ROWS])
            nc.vector.tensor_add(h1r[p][:, sp:ROWS], U[p][:, sp:ROWS],
                                 V[p][:, 0:s * BL])
            nc.vector.tensor_scalar(h1r[p], h1r[p], 0.0, None, MAX)
        return h1r

    def make_h1_f8(s):
        sp = (N - s) * BL
        scratch = [wpool.tile([PT, ROWS], BF16, tag=f"h1r_{p}", bufs=3,
                              name=f"h1s_{s}_{p}") for p in range(4)]
        h1f8 = wpool.tile([PT, 4 * ROWS], FP8, tag="h1f8", bufs=2,
                          name=f"h1f8_{s}")
        for p in range(4):
            nc.vector.tensor_add(scratch[p][:, 0:sp], U[p][:, 0:sp],
                                 V[p][:, s * BL:ROWS])
            nc.vector.tensor_add(scratch[p][:, sp:ROWS], U[p][:, sp:ROWS],
                                 V[p][:, 0:s * BL])
            nc.vector.tensor_scalar(h1f8[:, p * ROWS:(p + 1) * ROWS],
                                    scratch[p], 0.0, None, MAX)
        return h1f8

    # Phase-1 drains stay off the DVE (scalar for U, DVE copy for V) so the
    # DVE queue reaches the first rotations' h1 work early.
    h1_pre = {}
    layer("iW1", objsT_b,
          lambda m, c, ps: nc.scalar.activation(
              U[m][:, c * NT:(c + 1) * NT], ps, IDENT, bias=bias("ib1", m)))
    layer("iW1b", objsT_b,
          lambda m, c, ps: nc.vector.tensor_copy(V[m][:, c * NT:(c + 1) * NT], ps))
    h1_pre[1] = make_h1(1)
    layer_dr("tW1d", objs8,
             lambda m, c, ps: nc.scalar.activation(
                 t1f8[:, m * ROWS + c * NT:m * ROWS + (c + 1) * NT], ps, RELU,
                 bias=bias("tb1", m)))
    h1_pre[2] = make_h1(2)

    # ---- phase 2: rotation loop ----------------------------------------
    # Rotation s pairs every i with j = (i+s) mod 16 at once, skipping the
    # masked diagonal entirely. Column order everywhere is (i, b), matching
    # U, t3 and the output, so the j-sum becomes a running accumulation
    # acc += relu(L3 + ib3) with no masking or tree.
    acc = [cpool.tile([PT, ROWS], F32, tag=f"acc_{p}", bufs=1, name=f"acc_{p}")
           for p in range(2)]
    osb_pre = [cpool.tile([PT, ROWS], F32, tag=f"osb_pre_{p}", bufs=1,
                          name=f"osb_pre_{p}") for p in range(2)]
    for s in range(1, N):
        use_f8 = s > (N - 1 - NROT_F8)
        if s == N - 1:
            # acc holds s=1..14; fold in the residual early so the last
            # rotation's tail is just drain -> add -> store per quarter
            for p in range(2):
                nc.gpsimd.tensor_add(osb_pre[p], acc[p], t3[p])

        xT2 = [[wpool.tile([PT, NT], BF16, tag=f"xT2_{ih}_{m}", bufs=2,
                           name=f"xT2_{s}_{ih}_{m}") for m in range(4)]
               for ih in range(2)]
        if use_f8:
            h1f8 = make_h1_f8(s)
            r4 = h1f8.rearrange("p (kp h r) -> p kp h r", kp=2, h=2)
            for m in range(4):
                pss = [ppool.tile([PT, NT], F32, tag="ps", bufs=8,
                                  name=f"psL2_{s}_{ih}_{m}") for ih in range(2)]
                for kp in range(2):
                    for ih in range(2):
                        nc.tensor.matmul(pss[ih], wdr("iW2d", kp, m),
                                         r4[:, kp, :, ih * NT:(ih + 1) * NT],
                                         start=(kp == 0), stop=(kp == 1),
                                         perf_mode=DRMODE)
                for ih in range(2):
                    nc.scalar.activation(xT2[ih][m], pss[ih], RELU,
                                         bias=bias("ib2", m))
        else:
            h1r = h1_pre.pop(s) if s in h1_pre else make_h1(s)
            for m in range(4):
                pss = [ppool.tile([PT, NT], F32, tag="ps", bufs=8,
                                  name=f"psL2_{s}_{ih}_{m}") for ih in range(2)]
                for k in range(4):
                    for ih in range(2):
                        nc.tensor.matmul(pss[ih], wslice("iW2", k, m),
                                         h1r[k][:, ih * NT:(ih + 1) * NT],
                                         start=(k == 0), stop=(k == 3))
                for ih in range(2):
                    nc.scalar.activation(xT2[ih][m], pss[ih], RELU,
                                         bias=bias("ib2", m))
        # L3 + accumulate: s==1 drains straight into acc (scalar); later
        # rotations drain to a temp and gpsimd adds it into acc. In fp8
        # rotations the temp drain runs on DVE (biased relu tensor_scalar)
        # because the shorter PE window leaves scalar as the bottleneck.
        for m in range(2):
            pss3 = [ppool.tile([PT, NT], F32, tag="ps", bufs=8,
                               name=f"psL3_{s}_{ih}_{m}") for ih in range(2)]
            for k in range(4):
                for ih in range(2):
                    nc.tensor.matmul(pss3[ih], wslice("iW3", k, m),
                                     xT2[ih][k], start=(k == 0), stop=(k == 3))
            for ih in range(2):
                cs = slice(ih * NT, (ih + 1) * NT)
                if s == 1:
                    nc.scalar.activation(acc[m][:, cs], pss3[ih], RELU,
                                         bias=bias("ib3", m))
                elif s < N - 1:
                    tmp = wpool.tile([PT, NT], F32, tag=f"tmp_{ih}_{m}",
                                     bufs=2, name=f"tmp_{s}_{ih}_{m}")
                    if use_f8:
                        nc.vector.tensor_scalar(tmp, pss3[ih], bias("ib3", m),
                                                0.0, ADD, MAX)
                    else:
                        nc.scalar.activation(tmp, pss3[ih], RELU,
                                             bias=bias("ib3", m))
                    nc.gpsimd.tensor_add(acc[m][:, cs], acc[m][:, cs], tmp)
                else:
                    # last rotation: finish this (m, ih) quarter end-to-end
                    # (drain + one add against osb_pre + store)
                    tmp = wpool.tile([PT, NT], F32, tag=f"tmp_{ih}_{m}",
                                     bufs=2, name=f"tmp_{s}_{ih}_{m}")
                    nc.scalar.activation(tmp, pss3[ih], RELU,
                                         bias=bias("ib3", m))
                    osb = wpool.tile([PT, NT], F32, tag=f"osb_{ih}_{m}",
                                     bufs=1, name=f"osb_{ih}_{m}")
                    nc.vector.tensor_add(osb, tmp, osb_pre[m][:, cs])
                    nc.sync.dma_start(
                        out=prm["out"].rearrange(
                            "(h p) n b -> p h n b", h=2)[:, m, ih * 8:(ih + 1) * 8],
                        in_=osb.rearrange("p (n b) -> p n b", n=8))

        # t-MLP layers 2/3 ride the early rotations' scalar/PE slack
        if s == 1:
            layer_dr("tW2d", t1f8,
                     lambda m, c, ps: nc.scalar.activation(
                         t2f8[:, m * ROWS + c * NT:m * ROWS + (c + 1) * NT],
                         ps, RELU, bias=bias("tb2", m)))
        elif s == 3:
            layer_dr("tW3d", t2f8,
                     lambda m, c, ps: nc.scalar.activation(
                         t3[m][:, c * NT:(c + 1) * NT], ps, RELU,
                         bias=bias("tb3", m)))
        elif s == 4:
            # S = t3 + objs  (in place, gpsimd: DVE is on the h1 chain)
            for p in range(2):
                nc.gpsimd.tensor_add(t3[p], t3[p], objsT_f[p])


def build_nc(loop_iters=None):
    """loop_iters: if set, wrap the whole body in a hardware For_i loop that
    repeats it that many times (used only for timing measurements)."""
    nc = bacc.Bacc("TRN2", target_bir_lowering=False, debug=False)
    prm = {}
    prm["objs"] = nc.declare_dram_parameter("objs", [PT, 2 * ROWS], F32, isOutput=False)
    prm["objsb"] = nc.declare_dram_parameter("objsb", [PT, 2 * ROWS], BF16, isOutput=False)
    prm["objs8"] = nc.declare_dram_parameter("objs8", [PT, 2 * ROWS], FP8, isOutput=False)
    for wname, (fin, fout) in WEIGHT_SHAPES.items():
        nk = fin // PT
        prm[wname] = nc.declare_dram_parameter(wname, [PT, nk * fout], BF16, isOutput=False)
    for wname, (fin, fout) in DR_WEIGHT_SHAPES.items():
        nkp = fin // 256
        prm[wname] = nc.declare_dram_parameter(wname, [PT, nkp * 2 * fout], FP8, isOutput=False)
    prm["biases"] = nc.declare_dram_parameter("biases", [PT, 20], F32, isOutput=False)
    prm["out"] = nc.declare_dram_parameter("out", [D, N, BL], F32, isOutput=True)
    from contextlib import ExitStack
    with tile.TileContext(nc) as tc:
        if loop_iters is None:
            with ExitStack() as ctx:
                _build_body(nc, tc, prm, ctx)
        else:
            with tc.For_i(0, loop_iters, 1):
                with ExitStack() as ctx:
                    _build_body(nc, tc, prm, ctx)
    nc.compile()
    return nc


_CACHE = {}


def _get_nc():
    if "nc" not in _CACHE:
        _CACHE["nc"] = build_nc()
    return _CACHE["nc"]


def _pack_ktiles(w):
    """[fin, fout] -> [128, nk*fout] with k-tiles side by side."""
    fin, fout = w.shape
    nk = fin // PT
    return np.ascontiguousarray(
        w.reshape(nk, PT, fout).transpose(1, 0, 2).reshape(PT, nk * fout))


def _pack_dr(w, F8):
    """[fin, fout] -> [128, nkp*2*fout] fp8 DoubleRow layout:
    col = kp*2*fout + m*256 + h*128 + c; contracted feature = kp*256+h*128+p."""
    fin, fout = w.shape
    nkp = fin // 256
    nm = fout // PT
    out = np.zeros((PT, nkp * 2 * fout), dtype=F8)
    w8 = w.astype(F8)
    for kp in range(nkp):
        for m in range(nm):
            for h in range(2):
                f0 = kp * 256 + h * 128
                col = kp * 2 * fout + m * 256 + h * 128
                out[:, col:col + 128] = w8[f0:f0 + 128, m * PT:(m + 1) * PT]
    return out


def make_in_maps(inputs):
    import ml_dtypes
    BF = ml_dtypes.bfloat16
    F8 = ml_dtypes.float8_e4m3
    shared = {}
    for name in ("iW2", "iW3"):
        shared[name] = _pack_ktiles(
            np.asarray(inputs[name], dtype=np.float32)).astype(BF)
    iW1 = np.asarray(inputs["iW1"], dtype=np.float32)
    shared["iW1"] = _pack_ktiles(iW1[:D]).astype(BF)
    shared["iW1b"] = _pack_ktiles(iW1[D:]).astype(BF)
    for dst, src in (("tW1d", "tW1"), ("tW2d", "tW2"), ("tW3d", "tW3"),
                     ("iW2d", "iW2")):
        shared[dst] = _pack_dr(np.asarray(inputs[src], dtype=np.float32), F8)
    bcols = []
    for bname in BIAS_COLS:
        b = np.asarray(inputs[bname], dtype=np.float32)
        bcols.append(b.reshape(-1, PT).T)       # [128, nb]
    shared["biases"] = np.ascontiguousarray(np.concatenate(bcols, axis=1))
    objs = np.asarray(inputs["objs"], dtype=np.float32)
    in_maps = []
    for c in range(NCORES):
        m = dict(shared)
        sl = objs[:, c * BL:(c + 1) * BL, :]            # [N, BL, D]
        oT = sl.transpose(2, 0, 1).reshape(D, ROWS)     # [256, 1024]
        oP = np.ascontiguousarray(
            oT.reshape(2, PT, ROWS).transpose(1, 0, 2).reshape(PT, 2 * ROWS))
        m["objs"] = oP
        m["objsb"] = np.ascontiguousarray(oP.astype(BF))
        m["objs8"] = np.ascontiguousarray(oP.astype(F8))
        in_maps.append(m)
    return in_maps


def kernel(**inputs):
    nc = _get_nc()
    in_maps = make_in_maps(inputs)
    res = run_bass_kernel_spmd(nc, in_maps, list(range(NCORES)))
    outs = [res.results[c]["out"].transpose(1, 2, 0) for c in range(NCORES)]  # -> [N, BL, D]
    return np.concatenate(outs, axis=1)


# revision 6
# speedup vs baseline: 1.0621x; 1.0621x over previous
"""Trainium2 Bass kernel for the interaction-network GNN (nn_Physics_7370163880185).

Reference computation (per batch element b, objects i=0..15, D=256):
  trans  = MLP_t(objs)                              # 256->512->512->256, relu x3
  pair(i,j) = concat(objs_i, objs_j)                # [512]
  inter  = MLP_i(pair)                              # 512->512->512->256, relu x3
  out    = trans + sum_{j != i} inter(i,j) + objs

Sharding: data-parallel over batch B=512 across 8 cores (64 per core).

Kernel strategy (per core):
  * Feature-on-partition layout: activations are [feat, rows] with
    rows = (n, b) flattened; matmul(out, lhsT=W[k,m], rhs=xT[k, rows])
    chains layers without transposes.
  * Interaction layer 1 is split: concat(a,b) @ iW1 = a @ iW1[:256] + b @ iW1[256:],
    so U = objs @ iW1[:256] + ib1 and V = objs @ iW1[256:] are computed once on
    N*B rows; h1(i,j) = relu(U_i + V_j) is a broadcast add.
  * Phase 2 iterates over rotations s=1..15: rotation s computes
    inter(i, (i+s) mod 16) for all i at once in (i, b) column order, so the
    masked diagonal is never computed, h1 = relu(U + roll(V, s)) is two
    dense shifted adds (no broadcast), and the j-sum is a running
    accumulation acc += relu(L3 + ib3) (no masking, no tree reduce).
  * Mixed precision: U/V/L1 and the interaction L3 run in bf16 (fp32 PSUM).
    The transition MLP and the L2 matmuls of the last NROT_F8 rotations run
    in fp8-e4m3 with perf_mode=DoubleRow (2 k-tiles contracted per pass,
    ~2x per-matmul), keeping worst-case output error ~1.2e-2 vs the 2e-2
    gate (budget measured elementwise against the fp32 oracle).
  * Engine balance: DVE does the h1 adds + relu/fp8-convert and, in the
    fp8 rotations (where the PE window shrinks below the scalar drain
    time), the L3 drains via tensor_scalar(psum, +bias, max 0). The
    scalar engine drains everything else; gpsimd owns acc accumulation.
  * The t-MLP's 2nd/3rd layers are emitted inside the rotation stream
    (after rotations 1 and 3) where the scalar engine has slack; phase 1
    proper is only U/V/L1 and the t-MLP first layer.
  * Weights/biases are packed host-side; fp8 DoubleRow weights use the
    [128, kp*2*fout] layout with col = kp*2*fout + m*256 + h*128 + c where
    the contracted feature is f = kp*256 + h*128 + p.
"""

import numpy as np

import concourse.bass as bass
import concourse.mybir as mybir
import concourse.tile as tile
from concourse import bacc
from concourse.bass_utils import run_bass_kernel_spmd

N = 16
B = 512
D = 256
NCORES = 8
BL = B // NCORES          # 64 batch rows per core
ROWS = N * BL             # 1024 (n, b) rows per core
PT = 128                  # partition tile
NT = 512                  # matmul free-dim tile (one PSUM bank of fp32)

NROT_F8 = 8               # rotations (of 15) whose L2 runs in fp8 DoubleRow

F32 = mybir.dt.float32
BF16 = mybir.dt.bfloat16
FP8 = mybir.dt.float8e4
RELU = mybir.ActivationFunctionType.Relu
IDENT = mybir.ActivationFunctionType.Identity
ADD = mybir.AluOpType.add
MAX = mybir.AluOpType.max
DRMODE = mybir.MatmulPerfMode.DoubleRow

# bf16 weights, packed [128, nk*fout] host-side (k-tiles side by side)
WEIGHT_SHAPES = {
    "iW1": (256, 512), "iW1b": (256, 512),
    "iW2": (512, 512), "iW3": (512, 256),
}
# fp8 DoubleRow weights, packed [128, nkp*2*fout]
DR_WEIGHT_SHAPES = {
    "tW1d": (256, 512), "tW2d": (512, 512), "tW3d": (512, 256),
    "iW2d": (512, 512),
}
# biases packed into one [128, 20] tensor, in this column order
BIAS_COLS = {"tb1": (0, 4), "tb2": (4, 8), "tb3": (8, 10),
             "ib1": (10, 14), "ib2": (14, 18), "ib3": (18, 20)}


def _build_body(nc, tc, prm, ctx):
    cpool = ctx.enter_context(tc.tile_pool(name="const", bufs=1))
    wpool = ctx.enter_context(tc.tile_pool(name="work", bufs=2))
    ppool = ctx.enter_context(tc.tile_pool(name="psum", bufs=2, space="PSUM"))

    # ---- inputs: packed SBUF layout, per-k-tile DMAs (parallel rings),
    # in first-use order ---------------------------------------------------
    biases = cpool.tile([PT, 20], F32, tag="biases", bufs=1, name="biases")
    nc.sync.dma_start(out=biases, in_=prm["biases"][:, :])

    objsb = cpool.tile([PT, 2 * ROWS], BF16, tag="objsb", bufs=1, name="objsb")
    for p in range(2):
        nc.sync.dma_start(out=objsb[:, p * ROWS:(p + 1) * ROWS],
                          in_=prm["objsb"][:, p * ROWS:(p + 1) * ROWS])
    objsT_b = [objsb[:, p * ROWS:(p + 1) * ROWS] for p in range(2)]

    objs8 = cpool.tile([PT, 2 * ROWS], FP8, tag="objs8", bufs=1, name="objs8")
    nc.sync.dma_start(out=objs8, in_=prm["objs8"][:, :])

    w_sb = {}

    def load_weights(*names):
        for wname in names:
            if wname in WEIGHT_SHAPES:
                fin, fout = WEIGHT_SHAPES[wname]
                dt = BF16
                ncols = (fin // PT) * fout
            else:
                fin, fout = DR_WEIGHT_SHAPES[wname]
                dt = FP8
                ncols = (fin // 256) * 2 * fout
            t = cpool.tile([PT, ncols], dt, tag=wname, bufs=1, name=wname)
            for c0 in range(0, ncols, fout):
                nc.sync.dma_start(out=t[:, c0:c0 + fout],
                                  in_=prm[wname][:, c0:c0 + fout])
            w_sb[wname] = t

    load_weights("iW1", "iW1b", "tW1d", "iW2", "tW2d", "iW2d", "tW3d", "iW3")

    def wslice(wname, k, m):
        fout = WEIGHT_SHAPES[wname][1]
        return w_sb[wname][:, k * fout + m * PT:k * fout + (m + 1) * PT]

    def wdr(wname, kp, m):
        fout = DR_WEIGHT_SHAPES[wname][1]
        base = kp * 2 * fout + m * 256
        return w_sb[wname][:, base:base + 256].rearrange("p (h c) -> p h c", h=2)

    def bias(bname, m):
        lo, hi = BIAS_COLS[bname]
        assert lo + m < hi
        return biases[:, lo + m:lo + m + 1]

    objsf = cpool.tile([PT, 2 * ROWS], F32, tag="objsf", bufs=1, name="objsf")
    for p in range(2):
        nc.sync.dma_start(out=objsf[:, p * ROWS:(p + 1) * ROWS],
                          in_=prm["objs"][:, p * ROWS:(p + 1) * ROWS])
    objsT_f = [objsf[:, p * ROWS:(p + 1) * ROWS] for p in range(2)]

    # ---- generic bf16 dense layer over the full ROWS ------------------
    # Pairs of column-chunks share each 128-col weight tile so LDWEIGHTS
    # amortizes over 2 matmuls, and consecutive MMs hit different banks.
    def layer(wname, rhs, drain):
        fin, fout = WEIGHT_SHAPES[wname]
        nk = fin // PT
        nm = fout // PT
        ncols = rhs[0].shape[-1]
        mc = [(m, c) for m in range(nm) for c in range(ncols // NT)]
        for g in range(0, len(mc), 2):
            grp = mc[g:g + 2]
            pss = [ppool.tile([PT, NT], F32, tag="ps", bufs=8,
                              name=f"ps_{wname}_{m}_{c}") for (m, c) in grp]
            for k in range(nk):
                for (m, c), ps in zip(grp, pss):
                    nc.tensor.matmul(
                        ps, wslice(wname, k, m),
                        rhs[k][:, c * NT:(c + 1) * NT],
                        start=(k == 0), stop=(k == nk - 1))
            for (m, c), ps in zip(grp, pss):
                drain(m, c, ps)

    # ---- generic fp8 DoubleRow layer: rhs is a [PT, nkp*2*ROWS] fp8 tile
    # with k-major column layout (col = k*ROWS + r, k = kp*2 + h) ----------
    def layer_dr(wname, rhs, drain):
        fin, fout = DR_WEIGHT_SHAPES[wname]
        nkp = fin // 256
        nm = fout // PT
        r4 = rhs.rearrange("p (kp h r) -> p kp h r", kp=nkp, h=2)
        for m in range(nm):
            pss = [ppool.tile([PT, NT], F32, tag="ps", bufs=8,
                              name=f"ps_{wname}_{m}_{c}") for c in range(2)]
            for kp in range(nkp):
                for c in range(2):
                    nc.tensor.matmul(
                        pss[c], wdr(wname, kp, m),
                        r4[:, kp, :, c * NT:(c + 1) * NT],
                        start=(kp == 0), stop=(kp == nkp - 1),
                        perf_mode=DRMODE)
            for c in range(2):
                drain(m, c, pss[c])

    def persist(tag, n_tiles, dt=BF16, cols=ROWS):
        return [cpool.tile([PT, cols], dt, tag=f"{tag}_{m}", bufs=1, name=f"{tag}_{m}")
                for m in range(n_tiles)]

    # ---- phase 1: U, V (bf16) + trans MLP layer 1 (fp8 DR) ---------------
    U = persist("U", 4)       # bf16(objs @ iW1[:256] + ib1)
    V = persist("V", 4)       # bf16(objs @ iW1[256:])
    t1f8 = cpool.tile([PT, 4 * ROWS], FP8, tag="t1f8", bufs=1, name="t1f8")
    t2f8 = cpool.tile([PT, 4 * ROWS], FP8, tag="t2f8", bufs=1, name="t2f8")
    t3 = persist("t3", 2, F32)   # becomes S = t3 + objs after in-place add

    # h1 for rotation s: h1 = relu(U + roll(V, s)) — two dense shifted adds
    # plus a relu pass, all on DVE. bf16 rotations keep 4x [PT, ROWS] bf16
    # tiles; fp8 rotations write one [PT, 4*ROWS] fp8 tile (k-major).
    def make_h1(s):
        sp = (N - s) * BL     # split point: i < N-s reads V at +s*BL
        h1r = [wpool.tile([PT, ROWS], BF16, tag=f"h1r_{p}", bufs=3,
                          name=f"h1r_{s}_{p}") for p in range(4)]
        for p in range(4):
            nc.vector.tensor_add(h1r[p][:, 0:sp], U[p][:, 0:sp],
                                 V[p][:, s * BL:ROWS])
            nc.vector.tensor_add(h1r[p][:, sp:ROWS], U[p][:, sp:ROWS],
                                 V[p][:, 0:s * BL])
            nc.vector.tensor_scalar(h1r[p], h1r[p], 0.0, None, MAX)
        return h1r

    def make_h1_f8(s):
        sp = (N - s) * BL
        scratch = [wpool.tile([PT, ROWS], BF16, tag=f"h1r_{p}", bufs=3,
                              name=f"h1s_{s}_{p}") for p in range(4)]
        h1f8 = wpool.tile([PT, 4 * ROWS], FP8, tag="h1f8", bufs=2,
                          name=f"h1f8_{s}")
        for p in range(4):
            nc.vector.tensor_add(scratch[p][:, 0:sp], U[p][:, 0:sp],
                                 V[p][:, s * BL:ROWS])
            nc.vector.tensor_add(scratch[p][:, sp:ROWS], U[p][:, sp:ROWS],
                                 V[p][:, 0:s * BL])
            nc.vector.tensor_scalar(h1f8[:, p * ROWS:(p + 1) * ROWS],
                                    scratch[p], 0.0, None, MAX)
        return h1f8

    # Phase-1 drains stay off the DVE (scalar for U, DVE copy for V) so the
    # DVE queue reaches the first rotations' h1 work early.
    h1_pre = {}
    layer("iW1", objsT_b,
          lambda m, c, ps: nc.scalar.activation(
              U[m][:, c * NT:(c + 1) * NT], ps, IDENT, bias=bias("ib1", m)))
    layer("iW1b", objsT_b,
          lambda m, c, ps: nc.vector.tensor_copy(V[m][:, c * NT:(c + 1) * NT], ps))
    h1_pre[1] = make_h1(1)
    layer_dr("tW1d", objs8,
             lambda m, c, ps: nc.scalar.activation(
                 t1f8[:, m * ROWS + c * NT:m * ROWS + (c + 1) * NT], ps, RELU,
                 bias=bias("tb1", m)))
    h1_pre[2] = make_h1(2)

    # ---- phase 2: rotation loop ----------------------------------------
    # Rotation s pairs every i with j = (i+s) mod 16 at once, skipping the
    # masked diagonal entirely. Column order everywhere is (i, b), matching
    # U, t3 and the output, so the j-sum becomes a running accumulation
    # acc += relu(L3 + ib3) with no masking or tree.
    acc = [cpool.tile([PT, ROWS], F32, tag=f"acc_{p}", bufs=1, name=f"acc_{p}")
           for p in range(2)]
    osb_pre = [cpool.tile([PT, ROWS], F32, tag=f"osb_pre_{p}", bufs=1,
                          name=f"osb_pre_{p}") for p in range(2)]
    for s in range(1, N):
        use_f8 = s > (N - 1 - NROT_F8)
        # Software-pipeline h1: build rotation s+1's h1 NOW, ahead of this
        # rotation's DVE drains, so the DVE (strict in-order) always has the
        # next rotation's h1 ready before the PE reaches its L2 matmuls.
        if s + 1 < N and s + 1 not in h1_pre:
            if s + 1 > (N - 1 - NROT_F8):
                h1_pre[s + 1] = make_h1_f8(s + 1)
            else:
                h1_pre[s + 1] = make_h1(s + 1)
        if s == N - 1:
            # acc holds s=1..14; fold in the residual early so the last
            # rotation's tail is just drain -> add -> store per quarter
            for p in range(2):
                nc.gpsimd.tensor_add(osb_pre[p], acc[p], t3[p])

        xT2 = [[wpool.tile([PT, NT], BF16, tag=f"xT2_{ih}_{m}", bufs=2,
                           name=f"xT2_{s}_{ih}_{m}") for m in range(4)]
               for ih in range(2)]
        if use_f8:
            h1f8 = h1_pre.pop(s) if s in h1_pre else make_h1_f8(s)
            r4 = h1f8.rearrange("p (kp h r) -> p kp h r", kp=2, h=2)
            for m in range(4):
                pss = [ppool.tile([PT, NT], F32, tag="ps", bufs=8,
                                  name=f"psL2_{s}_{ih}_{m}") for ih in range(2)]
                for kp in range(2):
                    for ih in range(2):
                        nc.tensor.matmul(pss[ih], wdr("iW2d", kp, m),
                                         r4[:, kp, :, ih * NT:(ih + 1) * NT],
                                         start=(kp == 0), stop=(kp == 1),
                                         perf_mode=DRMODE)
                for ih in range(2):
                    nc.scalar.activation(xT2[ih][m], pss[ih], RELU,
                                         bias=bias("ib2", m))
        else:
            h1r = h1_pre.pop(s) if s in h1_pre else make_h1(s)
            for m in range(4):
                pss = [ppool.tile([PT, NT], F32, tag="ps", bufs=8,
                                  name=f"psL2_{s}_{ih}_{m}") for ih in range(2)]
                for k in range(4):
                    for ih in range(2):
                        nc.tensor.matmul(pss[ih], wslice("iW2", k, m),
                                         h1r[k][:, ih * NT:(ih + 1) * NT],
                                         start=(k == 0), stop=(k == 3))
                for ih in range(2):
                    nc.scalar.activation(xT2[ih][m], pss[ih], RELU,
                                         bias=bias("ib2", m))
        # L3 + accumulate: s==1 drains straight into acc (scalar); later
        # rotations drain to a temp and gpsimd adds it into acc. In fp8
        # rotations the temp drain runs on DVE (biased relu tensor_scalar)
        # because the shorter PE window leaves scalar as the bottleneck.
        for m in range(2):
            pss3 = [ppool.tile([PT, NT], F32, tag="ps", bufs=8,
                               name=f"psL3_{s}_{ih}_{m}") for ih in range(2)]
            for k in range(4):
                for ih in range(2):
                    nc.tensor.matmul(pss3[ih], wslice("iW3", k, m),
                                     xT2[ih][k], start=(k == 0), stop=(k == 3))
            for ih in range(2):
                cs = slice(ih * NT, (ih + 1) * NT)
                if s == 1:
                    nc.scalar.activation(acc[m][:, cs], pss3[ih], RELU,
                                         bias=bias("ib3", m))
                elif s < N - 1:
                    tmp = wpool.tile([PT, NT], F32, tag=f"tmp_{ih}_{m}",
                                     bufs=2, name=f"tmp_{s}_{ih}_{m}")
                    if use_f8:
                        nc.vector.tensor_scalar(tmp, pss3[ih], bias("ib3", m),
                                                0.0, ADD, MAX)
                    else:
                        nc.scalar.activation(tmp, pss3[ih], RELU,
                                             bias=bias("ib3", m))
                    nc.gpsimd.tensor_add(acc[m][:, cs], acc[m][:, cs], tmp)
                else:
                    # last rotation: finish this (m, ih) quarter end-to-end
                    # (drain + one add against osb_pre + store)
                    tmp = wpool.tile([PT, NT], F32, tag=f"tmp_{ih}_{m}",
                                     bufs=2, name=f"tmp_{s}_{ih}_{m}")
                    nc.scalar.activation(tmp, pss3[ih], RELU,
                                         bias=bias("ib3", m))
                    osb = wpool.tile([PT, NT], F32, tag=f"osb_{ih}_{m}",
                                     bufs=1, name=f"osb_{ih}_{m}")
                    nc.vector.tensor_add(osb, tmp, osb_pre[m][:, cs])
                    nc.sync.dma_start(
                        out=prm["out"].rearrange(
                            "(h p) n b -> p h n b", h=2)[:, m, ih * 8:(ih + 1) * 8],
                        in_=osb.rearrange("p (n b) -> p n b", n=8))

        # t-MLP layers 2/3 ride the early rotations' scalar/PE slack
        if s == 1:
            layer_dr("tW2d", t1f8,
                     lambda m, c, ps: nc.scalar.activation(
                         t2f8[:, m * ROWS + c * NT:m * ROWS + (c + 1) * NT],
                         ps, RELU, bias=bias("tb2", m)))
        elif s == 3:
            layer_dr("tW3d", t2f8,
                     lambda m, c, ps: nc.scalar.activation(
                         t3[m][:, c * NT:(c + 1) * NT], ps, RELU,
                         bias=bias("tb3", m)))
        elif s == 4:
            # S = t3 + objs  (in place, gpsimd: DVE is on the h1 chain)
            for p in range(2):
                nc.gpsimd.tensor_add(t3[p], t3[p], objsT_f[p])


def build_nc(loop_iters=None):
    """loop_iters: if set, wrap the whole body in a hardware For_i loop that
    repeats it that many times (used only for timing measurements)."""
    nc = bacc.Bacc("TRN2", target_bir_lowering=False, debug=False)
    prm = {}
    prm["objs"] = nc.declare_dram_parameter("objs", [PT, 2 * ROWS], F32, isOutput=False)
    prm["objsb"] = nc.declare_dram_parameter("objsb", [PT, 2 * ROWS], BF16, isOutput=False)
    prm["objs8"] = nc.declare_dram_parameter("objs8", [PT, 2 * ROWS], FP8, isOutput=False)
    for wname, (fin, fout) in WEIGHT_SHAPES.items():
        nk = fin // PT
        prm[wname] = nc.declare_dram_parameter(wname, [PT, nk * fout], BF16, isOutput=False)
    for wname, (fin, fout) in DR_WEIGHT_SHAPES.items():
        nkp = fin // 256
        prm[wname] = nc.declare_dram_parameter(wname, [PT, nkp * 2 * fout], FP8, isOutput=False)
    prm["biases"] = nc.declare_dram_parameter("biases", [PT, 20], F32, isOutput=False)
    prm["out"] = nc.declare_dram_parameter("out", [D, N, BL], F32, isOutput=True)
    from contextlib import ExitStack
    with tile.TileContext(nc) as tc:
        if loop_iters is None:
            with ExitStack() as ctx:
                _build_body(nc, tc, prm, ctx)
        else:
            with tc.For_i(0, loop_iters, 1):
                with ExitStack() as ctx:
                    _build_body(nc, tc, prm, ctx)
    nc.compile()
    return nc


_CACHE = {}


def _get_nc():
    if "nc" not in _CACHE:
        _CACHE["nc"] = build_nc()
    return _CACHE["nc"]


def _pack_ktiles(w):
    """[fin, fout] -> [128, nk*fout] with k-tiles side by side."""
    fin, fout = w.shape
    nk = fin // PT
    return np.ascontiguousarray(
        w.reshape(nk, PT, fout).transpose(1, 0, 2).reshape(PT, nk * fout))


def _pack_dr(w, F8):
    """[fin, fout] -> [128, nkp*2*fout] fp8 DoubleRow layout:
    col = kp*2*fout + m*256 + h*128 + c; contracted feature = kp*256+h*128+p."""
    fin, fout = w.shape
    nkp = fin // 256
    nm = fout // PT
    out = np.zeros((PT, nkp * 2 * fout), dtype=F8)
    w8 = w.astype(F8)
    for kp in range(nkp):
        for m in range(nm):
            for h in range(2):
                f0 = kp * 256 + h * 128
                col = kp * 2 * fout + m * 256 + h * 128
                out[:, col:col + 128] = w8[f0:f0 + 128, m * PT:(m + 1) * PT]
    return out


def make_in_maps(inputs):
    import ml_dtypes
    BF = ml_dtypes.bfloat16
    F8 = ml_dtypes.float8_e4m3
    shared = {}
    for name in ("iW2", "iW3"):
        shared[name] = _pack_ktiles(
            np.asarray(inputs[name], dtype=np.float32)).astype(BF)
    iW1 = np.asarray(inputs["iW1"], dtype=np.float32)
    shared["iW1"] = _pack_ktiles(iW1[:D]).astype(BF)
    shared["iW1b"] = _pack_ktiles(iW1[D:]).astype(BF)
    for dst, src in (("tW1d", "tW1"), ("tW2d", "tW2"), ("tW3d", "tW3"),
                     ("iW2d", "iW2")):
        shared[dst] = _pack_dr(np.asarray(inputs[src], dtype=np.float32), F8)
    bcols = []
    for bname in BIAS_COLS:
        b = np.asarray(inputs[bname], dtype=np.float32)
        bcols.append(b.reshape(-1, PT).T)       # [128, nb]
    shared["biases"] = np.ascontiguousarray(np.concatenate(bcols, axis=1))
    objs = np.asarray(inputs["objs"], dtype=np.float32)
    in_maps = []
    for c in range(NCORES):
        m = dict(shared)
        sl = objs[:, c * BL:(c + 1) * BL, :]            # [N, BL, D]
        oT = sl.transpose(2, 0, 1).reshape(D, ROWS)     # [256, 1024]
        oP = np.ascontiguousarray(
            oT.reshape(2, PT, ROWS).transpose(1, 0, 2).reshape(PT, 2 * ROWS))
        m["objs"] = oP
        m["objsb"] = np.ascontiguousarray(oP.astype(BF))
        m["objs8"] = np.ascontiguousarray(oP.astype(F8))
        in_maps.append(m)
    return in_maps


def kernel(**inputs):
    nc = _get_nc()
    in_maps = make_in_maps(inputs)
    res = run_bass_kernel_spmd(nc, in_maps, list(range(NCORES)))
    outs = [res.results[c]["out"].transpose(1, 2, 0) for c in range(NCORES)]  # -> [N, BL, D]
    return np.concatenate(outs, axis=1)
